# revision 18
# baseline (speedup 1.0000x reference)
"""Trainium2 Bass kernel for DynamicFilterWithImageInput.

Model (per batch b):
  img_feat = mean_hw(relu(BN1(conv2d(raw_img, w_conv1, 3x3, zeropad=1) + b1)))   # (64,)
  df       = softmax_over_C(BN2(img_feat @ w_filt.T + b_filt).reshape(C, K*K))   # (C, 25)
  out      = depthwise_conv5x5(reflect_pad(x_feat), df)                          # (C, H, W)

Sharding: pure data-parallel over batch (16 batches -> 8 cores x 2 batches).

Device mapping (per core, B_PC=2 batches):
  - conv1 as a single K=54 matmul (host-built im2col + block-diag weights,
    BN1 folded), ReLU+bias+spatial-sum via activation accum_out split over
    ScalarE/VectorE, mean+dense (K=65, bias row folded), softmax on
    [50, 256] (partition=(b,tap), free=channel).
  - depthwise 5x5 per slab (b, channel-group-of-128): 25 taps split
    between PE (diag-weight matmuls accumulating in PSUM, fp16) and
    VectorE (fused scalar_tensor_tensor psum += x * w[c]).  The last DVE
    tap writes SBUF (free PSUM evacuation), then DMA to HBM.
    PSUM processed in 4-bank quads (16 output rows) to amortize DVE
    per-op overhead.
"""

import os
import sys

sys.path.insert(0, "/opt/trn_rl_repo")

import numpy as np

import concourse.bass as bass
import concourse.bacc as bacc
import concourse.mybir as mybir
import concourse.tile as tile
from concourse.bass_utils import run_bass_kernel_spmd

F16 = mybir.dt.float16
F32 = mybir.dt.float32
AF = mybir.ActivationFunctionType
ALU = mybir.AluOpType

EPS = 1e-5
N_CORES = 8
B_PC = 2          # batches per core
C = 256           # channels
CG = C // 128     # channel groups of 128
K5 = 5            # depthwise kernel size
TAPS = [(i, j) for i in range(K5) for j in range(K5)]
NSLAB = B_PC * CG

_PROG_CACHE = {}


def _build_program(H, W, dve_splits=(7, 7)):
    """Emit the per-core Tile program. dve_splits: per-quad alternating
    number of DVE taps (rest on PE)."""
    Hp, Wp = H + 4, W + 4
    HWOUT = H * W
    GR = min(H, max(1, 512 // W))        # rows per matmul group (1 psum bank)
    QG = 4                               # matmul groups per quad (psum banks)
    QR = GR * QG                         # output rows per quad
    if H % QR != 0:
        QG = 1
        QR = GR
    assert H % QR == 0
    NQ = H // QR
    N1CH = min(512, HWOUT)               # conv1 psum chunk
    assert HWOUT % N1CH == 0
    N1 = HWOUT // N1CH                   # number of conv1 chunks

    for nd in dve_splits:
        assert 1 <= nd <= 24

    nc = bacc.Bacc("TRN2", target_bir_lowering=False, debug=False)

    x_d = nc.dram_tensor("x", [B_PC, C, Hp, Wp], F16, kind="ExternalInput").ap()
    im2col_d = nc.dram_tensor("im2col", [54, HWOUT], F16, kind="ExternalInput").ap()
    wconv_d = nc.dram_tensor("wconv", [54, 128], F16, kind="ExternalInput").ap()
    b1r_d = nc.dram_tensor("b1r", [128, 1], F32, kind="ExternalInput").ap()
    wft_d = nc.dram_tensor("wft", [65, C, 25], F16, kind="ExternalInput").ap()
    ident_d = nc.dram_tensor("ident", [128, 128], F16, kind="ExternalInput").ap()
    out_d = nc.dram_tensor("out", [B_PC, C, H, W], F32, kind="ExternalOutput").ap()

    # dram scratch for layout bounces
    imgf_d = nc.dram_tensor("imgf_sc", [128], F32).ap()
    df_d = nc.dram_tensor("df_sc", [B_PC, 25, C], F32).ap()
    wsm_d = nc.dram_tensor("wsm_sc", [B_PC, 25, C], F32).ap()

    with tile.TileContext(nc) as tc:
        with (
            tc.tile_pool(name="consts", bufs=1) as consts,
            tc.tile_pool(name="p0", bufs=1) as p0,
            tc.tile_pool(name="trash", bufs=2) as trashp,
            tc.tile_pool(name="xp", bufs=3) as xpp,
            tc.tile_pool(name="diag", bufs=50) as diagp,
            tc.tile_pool(name="ot", bufs=3) as otp,
            tc.tile_pool(name="psum", bufs=2, space="PSUM") as psump,
        ):
            # ---------- constants ----------
            im2col = consts.tile([54, HWOUT], F16)
            wconv = consts.tile([54, 128], F16)
            b1r = consts.tile([128, 1], F32)
            wft = consts.tile([65, C, 25], F16)
            ident = consts.tile([128, 128], F16)
            nc.sync.dma_start(wconv[:], wconv_d[:])
            nc.sync.dma_start(b1r[:], b1r_d[:])
            nc.sync.dma_start(wft[:], wft_d[:])
            nc.sync.dma_start(ident[:], ident_d[:])
            nc.sync.dma_start(im2col[:], im2col_d[:])

            # ---------- depthwise input loads (host pre-padded) ----------
            # Emit loads staggered: slabs 0..1 here, slab s+2 inside slab s's
            # loop, so no long-blocked DMA/ACT work sits ahead of ready work.
            xps = [None] * NSLAB

            def load_slab(s):
                b, cg = divmod(s, CG)
                xp = xpp.tile([128, Hp, Wp], F16, tag="xp")
                nc.sync.dma_start(xp[:], x_d[b, cg * 128:(cg + 1) * 128, :, :])
                xps[s] = xp

            for s in range(min(2, NSLAB)):
                load_slab(s)

            # ---------- phase 0: filter generation ----------
            acc = p0.tile([128, N1], F32)
            for ci in range(N1):
                ps1 = psump.tile([128, N1CH], F32, tag="ps")
                nc.tensor.matmul(
                    ps1[:], wconv[:], im2col[:, ci * N1CH:(ci + 1) * N1CH],
                    start=True, stop=True,
                )
                tr = trashp.tile([128, N1CH], F32, tag="tr")
                nc.scalar.activation(
                    tr[:], ps1[:], AF.Relu, bias=b1r[:], scale=1.0,
                    accum_out=acc[:, ci:ci + 1],
                )
            sfeat = p0.tile([128, 1], F32)
            if N1 > 1:
                nc.vector.tensor_reduce(sfeat[:], acc[:], mybir.AxisListType.X, ALU.add)
            else:
                nc.vector.tensor_copy(sfeat[:], acc[:])
            mfeat = p0.tile([128, 1], F32)
            nc.scalar.mul(mfeat[:], sfeat[:], 1.0 / HWOUT)
            nc.sync.dma_start(imgf_d[:], mfeat[:])

            # img_feat transposed [64,2] + ones row -> [65,2] fp16
            imgfT32 = p0.tile([65, B_PC], F32)
            nc.sync.dma_start(
                imgfT32[0:64, :],
                imgf_d[:].rearrange("(b o) -> o b", b=B_PC, o=64),
            )
            imgfT = p0.tile([65, B_PC], F16)
            nc.vector.tensor_copy(imgfT[0:64, :], imgfT32[0:64, :])
            nc.vector.memset(imgfT[64:65, :], 1.0)

            # dense: df[b, t, c] (+bias row), chunks of 2 taps
            for t0 in range(0, 25, 2):
                tw = min(2, 25 - t0)
                psd = psump.tile([B_PC, tw, C], F32, tag="ps")
                nc.tensor.matmul(
                    psd[:], imgfT[:],
                    wft[:, :, t0:t0 + tw].transpose([0, 2, 1]),
                    start=True, stop=True,
                )
                dfc = trashp.tile([B_PC, tw, C], F32, tag="dfc")
                nc.scalar.copy(dfc[:], psd[:])
                nc.sync.dma_start(df_d[:, t0:t0 + tw, :], dfc[:])

            # softmax over channels on [50, 256]
            dfsb = p0.tile([B_PC * 25, C], F32)
            nc.sync.dma_start(dfsb[:], df_d[:].flatten_outer_dims())
            edf = p0.tile([B_PC * 25, C], F32)
            nc.scalar.activation(edf[:], dfsb[:], AF.Exp)
            ssum = p0.tile([B_PC * 25, 1], F32)
            nc.vector.tensor_reduce(ssum[:], edf[:], mybir.AxisListType.X, ALU.add)
            rsum = p0.tile([B_PC * 25, 1], F32)
            nc.vector.reciprocal(rsum[:], ssum[:])
            wsm = p0.tile([B_PC * 25, C], F32)
            nc.vector.tensor_scalar(wsm[:], edf[:], rsum[:], None, ALU.mult)
            nc.sync.dma_start(wsm_d[:].flatten_outer_dims(), wsm[:])

            # per-slab filter values [128(c), 25] fp32
            vts = []
            for s in range(NSLAB):
                b, cg = divmod(s, CG)
                vt = p0.tile([128, 25], F32, tag="vt")
                nc.sync.dma_start(
                    vt[:], wsm_d[b, :, cg * 128:(cg + 1) * 128].transpose([1, 0])
                )
                vts.append(vt)

            # ---------- depthwise ----------
            qidx = 0
            for s in range(NSLAB):
                b, cg = divmod(s, CG)
                vt = vts[s]
                xp = xps[s]
                if s + 2 < NSLAB:
                    load_slab(s + 2)

                # per-tap diag tiles (fine-grained deps: PE can start
                # as soon as its first tap's diag is built)
                dts = []
                for t in range(25):
                    dt_ = diagp.tile([128, 128], F16, tag="dt")
                    nc.scalar.mul(dt_[:], ident[:], vt[:, t:t + 1])
                    dts.append(dt_)

                for q in range(NQ):
                    n_dve = dve_splits[qidx % len(dve_splits)]
                    qidx += 1
                    pe_taps = TAPS[: 25 - n_dve]
                    dve_taps = TAPS[25 - n_dve:]
                    y0 = q * QR
                    ps = psump.tile([128, QR, W], F32, tag="ps")
                    ot = otp.tile([128, QR, W], F32, tag="ot")
                    for g in range(QG):
                        gy = y0 + g * GR
                        for k, (i, j) in enumerate(pe_taps):
                            nc.tensor.matmul(
                                ps[:, g * GR:(g + 1) * GR, :],
                                dts[i * K5 + j][:],
                                xp[:, gy + i:gy + i + GR, j:j + W],
                                start=(k == 0),
                                stop=(k == len(pe_taps) - 1),
                            )
                    for k, (i, j) in enumerate(dve_taps):
                        last = k == len(dve_taps) - 1
                        nc.vector.scalar_tensor_tensor(
                            ot[:] if last else ps[:],
                            xp[:, y0 + i:y0 + i + QR, j:j + W],
                            vt[:, i * K5 + j:i * K5 + j + 1],
                            ps[:],
                            ALU.mult,
                            ALU.add,
                        )
                    nc.sync.dma_start(
                        out_d[b, cg * 128:(cg + 1) * 128, y0:y0 + QR, :], ot[:]
                    )

    nc.compile()
    return nc


def get_program(H, W, dve_splits=(7, 7)):
    key = (H, W, tuple(dve_splits))
    if key not in _PROG_CACHE:
        _PROG_CACHE[key] = _build_program(H, W, dve_splits)
    return _PROG_CACHE[key]


def host_prep(x_feat, raw_img, w_conv1, b_conv1, g1, beta1, m1, v1,
              w_filt, b_filt, g2, beta2, m2, v2):
    """Fold BN params, build im2col + packed weights; returns per-core in_maps."""
    B, Cc, H, W = x_feat.shape
    assert Cc == C
    n_cores = B // B_PC

    a1 = g1 / np.sqrt(v1 + EPS)
    w1f = (w_conv1 * a1[:, None, None, None]).astype(np.float32)   # (64,3,3,3)
    b1f = (b_conv1 - m1) * a1 + beta1                               # (64,)

    a2 = g2 / np.sqrt(v2 + EPS)
    wff = (w_filt * a2[:, None]).astype(np.float32)                 # (6400,64)
    bff = (b_filt - m2) * a2 + beta2                                # (6400,)

    # wft[k, c, t]: k<64 -> wff[c*25+t, k]; k=64 -> bias row
    wft = np.empty((65, C, 25), np.float32)
    wft[:64] = wff.reshape(C, 25, 64).transpose(2, 0, 1)
    wft[64] = bff.reshape(C, 25)
    wft16 = wft.astype(np.float16)

    b1r = np.tile(b1f, B_PC).reshape(128, 1).astype(np.float32)

    ident = np.eye(128, dtype=np.float16)

    xpad16 = np.pad(x_feat, ((0, 0), (0, 0), (2, 2), (2, 2)),
                    mode="reflect").astype(np.float16)

    # conv1 im2col, zero pad 1: [54, H*W] per core
    rawpad = np.pad(raw_img, ((0, 0), (0, 0), (1, 1), (1, 1))).astype(np.float32)

    # wconv[b*27 + (c*9+i*3+j), b*64+o] = w1f[o, c, i, j]
    wconv = np.zeros((54, 128), np.float32)
    w_flat = w1f.transpose(1, 2, 3, 0).reshape(27, 64)  # (c*9+i*3+j, o)
    for b in range(B_PC):
        wconv[b * 27:(b + 1) * 27, b * 64:(b + 1) * 64] = w_flat
    wconv16 = wconv.astype(np.float16)

    in_maps = []
    for core in range(n_cores):
        bs = core * B_PC
        im2col = np.empty((54, H * W), np.float32)
        for b in range(B_PC):
            for c in range(3):
                for i in range(3):
                    for j in range(3):
                        p = b * 27 + c * 9 + i * 3 + j
                        im2col[p] = rawpad[bs + b, c, i:i + H, j:j + W].reshape(-1)
        in_maps.append({
            "x": xpad16[bs:bs + B_PC],
            "im2col": im2col.astype(np.float16),
            "wconv": wconv16,
            "b1r": b1r,
            "wft": wft16,
            "ident": ident,
        })
    return in_maps


def run(inputs, trace=False, dve_splits=(7, 7)):
    x_feat = inputs["x_feat"]
    B, _, H, W = x_feat.shape
    nc = get_program(H, W, dve_splits)
    in_maps = host_prep(**inputs)
    n_cores = len(in_maps)
    res = run_bass_kernel_spmd(nc, in_maps, list(range(n_cores)), trace=trace)
    out = np.concatenate([r["out"] for r in res.results], axis=0)
    return out, res


def kernel(**inputs) -> np.ndarray:
    out, _ = run(inputs, trace=False)
    return out


# revision 20
# speedup vs baseline: 1.0001x; 1.0001x over previous
"""Trainium2 Bass kernel for DynamicFilterWithImageInput.

Model (per batch b):
  img_feat = mean_hw(relu(BN1(conv2d(raw_img, w_conv1, 3x3, zeropad=1) + b1)))   # (64,)
  df       = softmax_over_C(BN2(img_feat @ w_filt.T + b_filt).reshape(C, K*K))   # (C, 25)
  out      = depthwise_conv5x5(reflect_pad(x_feat), df)                          # (C, H, W)

Sharding: pure data-parallel over batch (16 batches -> 8 cores x 2 batches).

Device mapping (per core, B_PC=2 batches):
  - conv1 as a single K=54 matmul (host-built im2col + block-diag weights,
    BN1 folded), ReLU+bias+spatial-sum via activation accum_out split over
    ScalarE/VectorE, mean+dense (K=65, bias row folded), softmax on
    [50, 256] (partition=(b,tap), free=channel).
  - depthwise 5x5 per slab (b, channel-group-of-128): 25 taps split
    between PE (diag-weight matmuls accumulating in PSUM, fp16) and
    VectorE (fused scalar_tensor_tensor psum += x * w[c]).  The last DVE
    tap writes SBUF (free PSUM evacuation), then DMA to HBM.
    PSUM processed in 4-bank quads (16 output rows) to amortize DVE
    per-op overhead.
"""

import os
import sys

sys.path.insert(0, "/opt/trn_rl_repo")

import numpy as np

import concourse.bass as bass
import concourse.bacc as bacc
import concourse.mybir as mybir
import concourse.tile as tile
from concourse.bass_utils import run_bass_kernel_spmd

F16 = mybir.dt.float16
F32 = mybir.dt.float32
AF = mybir.ActivationFunctionType
ALU = mybir.AluOpType

EPS = 1e-5
N_CORES = 8
B_PC = 2          # batches per core
C = 256           # channels
CG = C // 128     # channel groups of 128
K5 = 5            # depthwise kernel size
TAPS = [(i, j) for i in range(K5) for j in range(K5)]
NSLAB = B_PC * CG

_PROG_CACHE = {}


def _build_program(H, W, dve_splits=(7, 7)):
    """Emit the per-core Tile program. dve_splits: per-quad alternating
    number of DVE taps (rest on PE)."""
    Hp, Wp = H + 4, W + 4
    HWOUT = H * W
    GR = min(H, max(1, 512 // W))        # rows per matmul group (1 psum bank)
    QG = 4                               # matmul groups per quad (psum banks)
    QR = GR * QG                         # output rows per quad
    if H % QR != 0:
        QG = 1
        QR = GR
    assert H % QR == 0
    NQ = H // QR
    N1CH = min(512, HWOUT)               # conv1 psum chunk
    assert HWOUT % N1CH == 0
    N1 = HWOUT // N1CH                   # number of conv1 chunks

    for nd in dve_splits:
        assert 1 <= nd <= 24

    nc = bacc.Bacc("TRN2", target_bir_lowering=False, debug=False)

    x_d = nc.dram_tensor("x", [B_PC, C, Hp, Wp], F16, kind="ExternalInput").ap()
    im2col_d = nc.dram_tensor("im2col", [54, HWOUT], F16, kind="ExternalInput").ap()
    wconv_d = nc.dram_tensor("wconv", [54, 128], F16, kind="ExternalInput").ap()
    b1r_d = nc.dram_tensor("b1r", [128, 1], F32, kind="ExternalInput").ap()
    wft_d = nc.dram_tensor("wft", [65, C, 25], F16, kind="ExternalInput").ap()
    ident_d = nc.dram_tensor("ident", [128, 128], F16, kind="ExternalInput").ap()
    out_d = nc.dram_tensor("out", [B_PC, C, H, W], F32, kind="ExternalOutput").ap()

    # dram scratch for layout bounces
    imgf_d = nc.dram_tensor("imgf_sc", [128], F32).ap()
    df_d = nc.dram_tensor("df_sc", [B_PC, 25, C], F32).ap()
    wsm_d = nc.dram_tensor("wsm_sc", [B_PC, 25, C], F32).ap()

    with tile.TileContext(nc) as tc:
        with (
            tc.tile_pool(name="consts", bufs=1) as consts,
            tc.tile_pool(name="p0", bufs=1) as p0,
            tc.tile_pool(name="trash", bufs=2) as trashp,
            tc.tile_pool(name="xp", bufs=3) as xpp,
            tc.tile_pool(name="diag", bufs=4 * 25) as diagp,
            tc.tile_pool(name="ot", bufs=3) as otp,
            tc.tile_pool(name="psum", bufs=2, space="PSUM") as psump,
        ):
            # ---------- constants ----------
            im2col = consts.tile([54, HWOUT], F16)
            wconv = consts.tile([54, 128], F16)
            b1r = consts.tile([128, 1], F32)
            wft = consts.tile([65, C, 25], F16)
            ident = consts.tile([128, 128], F16)
            nc.sync.dma_start(wconv[:], wconv_d[:])
            nc.sync.dma_start(b1r[:], b1r_d[:])
            nc.sync.dma_start(wft[:], wft_d[:])
            nc.sync.dma_start(ident[:], ident_d[:])
            nc.sync.dma_start(im2col[:], im2col_d[:])

            # ---------- depthwise input loads (host pre-padded) ----------
            # Emit loads staggered: slabs 0..1 here, slab s+2 inside slab s's
            # loop, so no long-blocked DMA/ACT work sits ahead of ready work.
            xps = [None] * NSLAB

            def load_slab(s):
                b, cg = divmod(s, CG)
                xp = xpp.tile([128, Hp, Wp], F16, tag="xp")
                nc.sync.dma_start(xp[:], x_d[b, cg * 128:(cg + 1) * 128, :, :])
                xps[s] = xp

            for s in range(min(2, NSLAB)):
                load_slab(s)

            # ---------- phase 0: filter generation ----------
            acc = p0.tile([128, N1], F32)
            for ci in range(N1):
                ps1 = psump.tile([128, N1CH], F32, tag="ps")
                nc.tensor.matmul(
                    ps1[:], wconv[:], im2col[:, ci * N1CH:(ci + 1) * N1CH],
                    start=True, stop=True,
                )
                tr = trashp.tile([128, N1CH], F32, tag="tr")
                nc.scalar.activation(
                    tr[:], ps1[:], AF.Relu, bias=b1r[:], scale=1.0,
                    accum_out=acc[:, ci:ci + 1],
                )
            sfeat = p0.tile([128, 1], F32)
            if N1 > 1:
                nc.vector.tensor_reduce(sfeat[:], acc[:], mybir.AxisListType.X, ALU.add)
            else:
                nc.vector.tensor_copy(sfeat[:], acc[:])
            mfeat = p0.tile([128, 1], F32)
            nc.scalar.mul(mfeat[:], sfeat[:], 1.0 / HWOUT)
            nc.sync.dma_start(imgf_d[:], mfeat[:])

            # img_feat transposed [64,2] + ones row -> [65,2] fp16
            imgfT32 = p0.tile([65, B_PC], F32)
            nc.sync.dma_start(
                imgfT32[0:64, :],
                imgf_d[:].rearrange("(b o) -> o b", b=B_PC, o=64),
            )
            imgfT = p0.tile([65, B_PC], F16)
            nc.vector.tensor_copy(imgfT[0:64, :], imgfT32[0:64, :])
            nc.vector.memset(imgfT[64:65, :], 1.0)

            # dense: df[b, t, c] (+bias row), chunks of 2 taps
            for t0 in range(0, 25, 2):
                tw = min(2, 25 - t0)
                psd = psump.tile([B_PC, tw, C], F32, tag="ps")
                nc.tensor.matmul(
                    psd[:], imgfT[:],
                    wft[:, :, t0:t0 + tw].transpose([0, 2, 1]),
                    start=True, stop=True,
                )
                dfc = trashp.tile([B_PC, tw, C], F32, tag="dfc")
                nc.scalar.copy(dfc[:], psd[:])
                nc.sync.dma_start(df_d[:, t0:t0 + tw, :], dfc[:])

            # softmax over channels on [50, 256]
            dfsb = p0.tile([B_PC * 25, C], F32)
            nc.sync.dma_start(dfsb[:], df_d[:].flatten_outer_dims())
            edf = p0.tile([B_PC * 25, C], F32)
            nc.scalar.activation(edf[:], dfsb[:], AF.Exp)
            ssum = p0.tile([B_PC * 25, 1], F32)
            nc.vector.tensor_reduce(ssum[:], edf[:], mybir.AxisListType.X, ALU.add)
            rsum = p0.tile([B_PC * 25, 1], F32)
            nc.vector.reciprocal(rsum[:], ssum[:])
            wsm = p0.tile([B_PC * 25, C], F32)
            nc.vector.tensor_scalar(wsm[:], edf[:], rsum[:], None, ALU.mult)
            nc.sync.dma_start(wsm_d[:].flatten_outer_dims(), wsm[:])

            # per-slab filter values [128(c), 25] fp32
            vts = []
            for s in range(NSLAB):
                b, cg = divmod(s, CG)
                vt = p0.tile([128, 25], F32, tag="vt")
                nc.sync.dma_start(
                    vt[:], wsm_d[b, :, cg * 128:(cg + 1) * 128].transpose([1, 0])
                )
                vts.append(vt)

            # all diag tiles up-front (ACT runs them alongside slab 0; keeps
            # slab boundaries free of ACT head-of-line stalls)
            dts_all = []
            for s in range(NSLAB):
                dts = []
                for t in range(25):
                    dt_ = diagp.tile([128, 128], F16, tag="dt")
                    nc.scalar.mul(dt_[:], ident[:], vts[s][:, t:t + 1])
                    dts.append(dt_)
                dts_all.append(dts)

            # ---------- depthwise ----------
            qidx = 0
            for s in range(NSLAB):
                b, cg = divmod(s, CG)
                vt = vts[s]
                xp = xps[s]
                dts = dts_all[s]
                if s + 2 < NSLAB:
                    load_slab(s + 2)

                for q in range(NQ):
                    n_dve = dve_splits[qidx % len(dve_splits)]
                    qidx += 1
                    pe_taps = TAPS[: 25 - n_dve]
                    dve_taps = TAPS[25 - n_dve:]
                    y0 = q * QR
                    ps = psump.tile([128, QR, W], F32, tag="ps")
                    ot = otp.tile([128, QR, W], F32, tag="ot")
                    for g in range(QG):
                        gy = y0 + g * GR
                        for k, (i, j) in enumerate(pe_taps):
                            nc.tensor.matmul(
                                ps[:, g * GR:(g + 1) * GR, :],
                                dts[i * K5 + j][:],
                                xp[:, gy + i:gy + i + GR, j:j + W],
                                start=(k == 0),
                                stop=(k == len(pe_taps) - 1),
                            )
                    for k, (i, j) in enumerate(dve_taps):
                        last = k == len(dve_taps) - 1
                        nc.vector.scalar_tensor_tensor(
                            ot[:] if last else ps[:],
                            xp[:, y0 + i:y0 + i + QR, j:j + W],
                            vt[:, i * K5 + j:i * K5 + j + 1],
                            ps[:],
                            ALU.mult,
                            ALU.add,
                        )
                    nc.sync.dma_start(
                        out_d[b, cg * 128:(cg + 1) * 128, y0:y0 + QR, :], ot[:]
                    )

    nc.compile()
    return nc


def get_program(H, W, dve_splits=(7, 7)):
    key = (H, W, tuple(dve_splits))
    if key not in _PROG_CACHE:
        _PROG_CACHE[key] = _build_program(H, W, dve_splits)
    return _PROG_CACHE[key]


def host_prep(x_feat, raw_img, w_conv1, b_conv1, g1, beta1, m1, v1,
              w_filt, b_filt, g2, beta2, m2, v2):
    """Fold BN params, build im2col + packed weights; returns per-core in_maps."""
    B, Cc, H, W = x_feat.shape
    assert Cc == C
    n_cores = B // B_PC

    a1 = g1 / np.sqrt(v1 + EPS)
    w1f = (w_conv1 * a1[:, None, None, None]).astype(np.float32)   # (64,3,3,3)
    b1f = (b_conv1 - m1) * a1 + beta1                               # (64,)

    a2 = g2 / np.sqrt(v2 + EPS)
    wff = (w_filt * a2[:, None]).astype(np.float32)                 # (6400,64)
    bff = (b_filt - m2) * a2 + beta2                                # (6400,)

    # wft[k, c, t]: k<64 -> wff[c*25+t, k]; k=64 -> bias row
    wft = np.empty((65, C, 25), np.float32)
    wft[:64] = wff.reshape(C, 25, 64).transpose(2, 0, 1)
    wft[64] = bff.reshape(C, 25)
    wft16 = wft.astype(np.float16)

    b1r = np.tile(b1f, B_PC).reshape(128, 1).astype(np.float32)

    ident = np.eye(128, dtype=np.float16)

    xpad16 = np.pad(x_feat, ((0, 0), (0, 0), (2, 2), (2, 2)),
                    mode="reflect").astype(np.float16)

    # conv1 im2col, zero pad 1: [54, H*W] per core
    rawpad = np.pad(raw_img, ((0, 0), (0, 0), (1, 1), (1, 1))).astype(np.float32)

    # wconv[b*27 + (c*9+i*3+j), b*64+o] = w1f[o, c, i, j]
    wconv = np.zeros((54, 128), np.float32)
    w_flat = w1f.transpose(1, 2, 3, 0).reshape(27, 64)  # (c*9+i*3+j, o)
    for b in range(B_PC):
        wconv[b * 27:(b + 1) * 27, b * 64:(b + 1) * 64] = w_flat
    wconv16 = wconv.astype(np.float16)

    in_maps = []
    for core in range(n_cores):
        bs = core * B_PC
        im2col = np.empty((54, H * W), np.float32)
        for b in range(B_PC):
            for c in range(3):
                for i in range(3):
                    for j in range(3):
                        p = b * 27 + c * 9 + i * 3 + j
                        im2col[p] = rawpad[bs + b, c, i:i + H, j:j + W].reshape(-1)
        in_maps.append({
            "x": xpad16[bs:bs + B_PC],
            "im2col": im2col.astype(np.float16),
            "wconv": wconv16,
            "b1r": b1r,
            "wft": wft16,
            "ident": ident,
        })
    return in_maps


def run(inputs, trace=False, dve_splits=(7, 7)):
    x_feat = inputs["x_feat"]
    B, _, H, W = x_feat.shape
    nc = get_program(H, W, dve_splits)
    in_maps = host_prep(**inputs)
    n_cores = len(in_maps)
    res = run_bass_kernel_spmd(nc, in_maps, list(range(n_cores)), trace=trace)
    out = np.concatenate([r["out"] for r in res.results], axis=0)
    return out, res


def kernel(**inputs) -> np.ndarray:
    out, _ = run(inputs, trace=False)
    return out


# revision 27
# speedup vs baseline: 1.1108x; 1.1106x over previous
"""Trainium2 Bass kernel for DynamicFilterWithImageInput.

Model (per batch b):
  img_feat = mean_hw(relu(BN1(conv2d(raw_img, w_conv1, 3x3, zeropad=1) + b1)))   # (64,)
  df       = softmax_over_C(BN2(img_feat @ w_filt.T + b_filt).reshape(C, K*K))   # (C, 25)
  out      = depthwise_conv5x5(reflect_pad(x_feat), df)                          # (C, H, W)

Sharding: pure data-parallel over batch (16 batches -> 8 cores x 2 batches).

Device mapping (per core, B_PC=2 batches):
  - conv1 as a single K=54 matmul (host-built im2col + block-diag weights,
    BN1 folded), ReLU+bias+spatial-sum via accum_out alternating
    ScalarE/VectorE, mean+dense (K=65, bias row folded), softmax on
    [50, 256] (partition=(b,tap), free=channel), filter transpose via PE.
  - depthwise 5x5 per slab (b, channel-group-of-128): 25 taps split across
    PE (diag-weight fp16 matmuls accumulating in PSUM), VectorE (fused
    scalar_tensor_tensor psum += x * w[c]) and GpSimdE (fp16 SBUF
    accumulator chain).  A final VectorE op merges psum + gpsimd partial
    into an SBUF tile (free PSUM evacuation), then DMA to HBM.
    PSUM processed in 4-bank quads (16 output rows) to amortize
    per-op overheads.  x arrives host-side reflect-padded in fp16.
"""

import os
import sys

sys.path.insert(0, "/opt/trn_rl_repo")

import numpy as np

import concourse.bass as bass
import concourse.bacc as bacc
import concourse.mybir as mybir
import concourse.tile as tile
from concourse.bass_utils import run_bass_kernel_spmd

F16 = mybir.dt.float16
F32 = mybir.dt.float32
AF = mybir.ActivationFunctionType
ALU = mybir.AluOpType

EPS = 1e-5
B_PC = 2          # batches per core
C = 256           # channels
CG = C // 128     # channel groups of 128
K5 = 5            # depthwise kernel size
TAPS = [(i, j) for i in range(K5) for j in range(K5)]
NSLAB = B_PC * CG

_PROG_CACHE = {}


def _build_program(H, W, dve_splits=(7, 6), n_gp=0):
    """Emit the per-core Tile program. Per quad (alternating over
    dve_splits): n_dve taps on VectorE, n_gp on GpSimd (+1 DVE merge),
    rest on PE."""
    Hp, Wp = H + 4, W + 4
    HWOUT = H * W
    GR = min(H, max(1, 512 // W))        # rows per matmul group (1 psum bank)
    QG = 4                               # matmul groups per quad (psum banks)
    QR = GR * QG                         # output rows per quad
    if H % QR != 0:
        QG = 1
        QR = GR
    assert H % QR == 0
    NQ = H // QR
    N1CH = min(512, HWOUT)               # conv1 psum chunk
    assert HWOUT % N1CH == 0
    N1 = HWOUT // N1CH                   # number of conv1 chunks
    IMCH = min(8 * N1CH, HWOUT)          # im2col streaming chunk


    nc = bacc.Bacc("TRN2", target_bir_lowering=False, debug=False)

    x_d = nc.dram_tensor("x", [B_PC, C, Hp, Wp], F16, kind="ExternalInput").ap()
    im2col_d = nc.dram_tensor("im2col", [54, HWOUT], F16, kind="ExternalInput").ap()
    wconv_d = nc.dram_tensor("wconv", [54, 128], F16, kind="ExternalInput").ap()
    b1r_d = nc.dram_tensor("b1r", [128, 1], F32, kind="ExternalInput").ap()
    wft_d = nc.dram_tensor("wft", [65, C, 25], F16, kind="ExternalInput").ap()
    ident_d = nc.dram_tensor("ident", [128, 128], F16, kind="ExternalInput").ap()
    id32_d = nc.dram_tensor("id32", [128, 128], F32, kind="ExternalInput").ap()
    out_d = nc.dram_tensor("out", [B_PC, C, H, W], F32, kind="ExternalOutput").ap()

    # dram scratch for layout bounces
    imgf_d = nc.dram_tensor("imgf_sc", [128], F32).ap()
    df_d = nc.dram_tensor("df_sc", [B_PC, 25, C], F32).ap()

    with tile.TileContext(nc) as tc:
        with (
            tc.tile_pool(name="consts", bufs=1) as consts,
            tc.tile_pool(name="p0", bufs=1) as p0,
            tc.tile_pool(name="imc", bufs=2) as imcp,
            tc.tile_pool(name="trash", bufs=2) as trashp,
            tc.tile_pool(name="xp", bufs=3) as xpp,
            tc.tile_pool(name="diag", bufs=NSLAB * 25) as diagp,
            tc.tile_pool(name="ot", bufs=2) as otp,
            tc.tile_pool(name="gacc", bufs=2) as gaccp,
            tc.tile_pool(name="psum", bufs=2, space="PSUM") as psump,
        ):
            # ---------- phase-0 constants (emitted first: queue priority) ----
            wconv = consts.tile([54, 128], F16)
            b1r = consts.tile([128, 1], F32)
            wft = consts.tile([65, C, 25], F16)
            ident = consts.tile([128, 128], F16)
            id32 = consts.tile([128, 128], F32)
            zeros = consts.tile([128, N1CH], F32)
            nc.sync.dma_start(wconv[:], wconv_d[:])
            nc.sync.dma_start(b1r[:], b1r_d[:])
            nc.sync.dma_start(wft[:], wft_d[:])
            nc.sync.dma_start(ident[:], ident_d[:])
            nc.sync.dma_start(id32[:], id32_d[:])
            nc.gpsimd.memset(zeros[:], 0.0)

            # ---------- conv1: streamed im2col, relu+sum split ACT/DVE ------
            acc = p0.tile([128, N1], F32)
            imt = None
            for ci in range(N1):
                if ci % (IMCH // N1CH) == 0:
                    imt = imcp.tile([54, IMCH], F16, tag="imc")
                    o0 = ci * N1CH
                    nc.sync.dma_start(imt[:], im2col_d[:, o0:o0 + IMCH])
                ps1 = psump.tile([128, N1CH], F32, tag="ps")
                off = (ci % (IMCH // N1CH)) * N1CH
                nc.tensor.matmul(
                    ps1[:], wconv[:], imt[:, off:off + N1CH],
                    start=True, stop=True,
                )
                tr = trashp.tile([128, N1CH], F32, tag="tr")
                if ci % 2 == 0:
                    nc.scalar.activation(
                        tr[:], ps1[:], AF.Relu, bias=b1r[:], scale=1.0,
                        accum_out=acc[:, ci:ci + 1],
                    )
                else:
                    nc.vector.scalar_tensor_tensor(
                        tr[:], ps1[:], b1r[:], zeros[:], ALU.add, ALU.max,
                        accum_out=acc[:, ci:ci + 1],
                    )
            sfeat = p0.tile([128, 1], F32)
            if N1 > 1:
                nc.vector.tensor_reduce(sfeat[:], acc[:], mybir.AxisListType.X, ALU.add)
            else:
                nc.vector.tensor_copy(sfeat[:], acc[:])
            mfeat = p0.tile([128, 1], F32)
            nc.scalar.mul(mfeat[:], sfeat[:], 1.0 / HWOUT)
            nc.sync.dma_start(imgf_d[:], mfeat[:])

            # img_feat transposed [64,2] + ones row -> [65,2] fp16
            imgfT32 = p0.tile([65, B_PC], F32)
            nc.sync.dma_start(
                imgfT32[0:64, :],
                imgf_d[:].rearrange("(b o) -> o b", b=B_PC, o=64),
            )
            imgfT = p0.tile([65, B_PC], F16)
            nc.vector.tensor_copy(imgfT[0:64, :], imgfT32[0:64, :])
            nc.vector.memset(imgfT[64:65, :], 1.0)

            # dense: df[b, t, c] (+bias row); 4 x 512 chunks per psum slot
            CH_PER_SLOT = 4
            t0 = 0
            while t0 < 25:
                tws = []
                t1 = t0
                while t1 < 25 and len(tws) < CH_PER_SLOT:
                    tw = min(2, 25 - t1)
                    tws.append((t1, tw))
                    t1 += tw
                tot = t1 - t0
                psd = psump.tile([B_PC, tot, C], F32, tag="ps")
                for (tt, tw) in tws:
                    nc.tensor.matmul(
                        psd[:, tt - t0:tt - t0 + tw, :], imgfT[:],
                        wft[:, :, tt:tt + tw].transpose([0, 2, 1]),
                        start=True, stop=True,
                    )
                dfc = trashp.tile([B_PC, tot, C], F32, tag="dfc")
                nc.scalar.copy(dfc[:], psd[:])
                nc.sync.dma_start(df_d[:, t0:t0 + tot, :], dfc[:])
                t0 = t1

            # softmax over channels; batch b parked at partition b*32 so the
            # PE transpose below sees base partitions in {0, 32}
            dfsb = p0.tile([B_PC * 32, C], F32)
            edf = p0.tile([B_PC * 32, C], F32)
            ssum = p0.tile([B_PC * 32, 1], F32)
            rsum = p0.tile([B_PC * 32, 1], F32)
            wsm = p0.tile([B_PC * 32, C], F32)
            for b in range(B_PC):
                sl = slice(b * 32, b * 32 + 25)
                nc.sync.dma_start(dfsb[sl, :], df_d[b])
                nc.scalar.activation(edf[sl, :], dfsb[sl, :], AF.Exp)
                nc.vector.tensor_reduce(
                    ssum[sl, :], edf[sl, :], mybir.AxisListType.X, ALU.add)
                nc.vector.reciprocal(rsum[sl, :], ssum[sl, :])
                nc.vector.tensor_scalar(
                    wsm[sl, :], edf[sl, :], rsum[sl, :], None, ALU.mult)

            # per-slab filter values [128(c), 25] via PE transpose (no bounce)
            vts = []
            for s in range(NSLAB):
                b, cg = divmod(s, CG)
                pst = psump.tile([128, 25], F32, tag="ps")
                nc.tensor.transpose(
                    pst[:], wsm[b * 32:b * 32 + 25, cg * 128:(cg + 1) * 128],
                    id32[b * 32:b * 32 + 25, 0:25],
                )
                vt = p0.tile([128, 25], F32, tag=f"vt{s}")
                nc.scalar.copy(vt[:], pst[:])
                vts.append(vt)

            # ---------- depthwise input loads (host pre-padded) -------------
            xps = [None] * NSLAB

            def load_slab(s):
                b, cg = divmod(s, CG)
                xp = xpp.tile([128, Hp, Wp], F16, tag="xp")
                nc.sync.dma_start(xp[:], x_d[b, cg * 128:(cg + 1) * 128, :, :])
                xps[s] = xp

            for s in range(min(2, NSLAB)):
                load_slab(s)

            # all diag tiles up-front (ACT churns through them early)
            dts_all = []
            for s in range(NSLAB):
                dts = []
                for t in range(25):
                    dt_ = diagp.tile([128, 128], F16, tag="dt")
                    nc.scalar.mul(dt_[:], ident[:], vts[s][:, t:t + 1])
                    dts.append(dt_)
                dts_all.append(dts)

            # ---------- depthwise ----------
            qidx = 0
            for s in range(NSLAB):
                b, cg = divmod(s, CG)
                vt = vts[s]
                xp = xps[s]
                dts = dts_all[s]
                if s + 2 < NSLAB:
                    load_slab(s + 2)

                for q in range(NQ):
                    n_dve = dve_splits[qidx % len(dve_splits)]
                    qidx += 1
                    n_pe = 25 - n_dve - n_gp
                    pe_taps = TAPS[:n_pe]
                    dve_taps = TAPS[n_pe:n_pe + n_dve]
                    gp_taps = TAPS[n_pe + n_dve:]
                    y0 = q * QR
                    ps = psump.tile([128, QR, W], F32, tag="ps")
                    ot = otp.tile([128, QR, W], F32, tag="ot")
                    for g in range(QG):
                        gy = y0 + g * GR
                        for k, (i, j) in enumerate(pe_taps):
                            nc.tensor.matmul(
                                ps[:, g * GR:(g + 1) * GR, :],
                                dts[i * K5 + j][:],
                                xp[:, gy + i:gy + i + GR, j:j + W],
                                start=(k == 0),
                                stop=(k == len(pe_taps) - 1),
                            )
                    # gpsimd partial chain (fp16 SBUF accumulator)
                    if n_gp:
                        ga = gaccp.tile([128, QR, W], F16, tag="ga")
                        for k, (i, j) in enumerate(gp_taps):
                            sc = vt[:, i * K5 + j:i * K5 + j + 1]
                            win = xp[:, y0 + i:y0 + i + QR, j:j + W]
                            if k == 0:
                                nc.gpsimd.tensor_scalar(
                                    ga[:], win, sc, None, ALU.mult)
                            else:
                                nc.gpsimd.scalar_tensor_tensor(
                                    ga[:], win, sc, ga[:], ALU.mult, ALU.add)
                    # DVE taps into psum, then merge
                    for k, (i, j) in enumerate(dve_taps):
                        is_merge_tap = (k == len(dve_taps) - 1) and n_gp == 0
                        nc.vector.scalar_tensor_tensor(
                            ot[:] if is_merge_tap else ps[:],
                            xp[:, y0 + i:y0 + i + QR, j:j + W],
                            vt[:, i * K5 + j:i * K5 + j + 1],
                            ps[:],
                            ALU.mult,
                            ALU.add,
                        )
                    if n_gp:
                        nc.vector.scalar_tensor_tensor(
                            ot[:], ga[:], 1.0, ps[:], ALU.mult, ALU.add)
                    nc.sync.dma_start(
                        out_d[b, cg * 128:(cg + 1) * 128, y0:y0 + QR, :], ot[:]
                    )

    nc.compile()
    return nc


def get_program(H, W, dve_splits=(7, 6), n_gp=0):
    key = (H, W, tuple(dve_splits), n_gp)
    if key not in _PROG_CACHE:
        _PROG_CACHE[key] = _build_program(H, W, dve_splits, n_gp)
    return _PROG_CACHE[key]


def host_prep(x_feat, raw_img, w_conv1, b_conv1, g1, beta1, m1, v1,
              w_filt, b_filt, g2, beta2, m2, v2):
    """Fold BN params, build im2col + packed weights; returns per-core in_maps."""
    B, Cc, H, W = x_feat.shape
    assert Cc == C
    n_cores = B // B_PC

    a1 = g1 / np.sqrt(v1 + EPS)
    w1f = (w_conv1 * a1[:, None, None, None]).astype(np.float32)   # (64,3,3,3)
    b1f = (b_conv1 - m1) * a1 + beta1                               # (64,)

    a2 = g2 / np.sqrt(v2 + EPS)
    wff = (w_filt * a2[:, None]).astype(np.float32)                 # (6400,64)
    bff = (b_filt - m2) * a2 + beta2                                # (6400,)

    # wft[k, c, t]: k<64 -> wff[c*25+t, k]; k=64 -> bias row
    wft = np.empty((65, C, 25), np.float32)
    wft[:64] = wff.reshape(C, 25, 64).transpose(2, 0, 1)
    wft[64] = bff.reshape(C, 25)
    wft16 = wft.astype(np.float16)

    b1r = np.tile(b1f, B_PC).reshape(128, 1).astype(np.float32)

    ident = np.eye(128, dtype=np.float16)
    # 25x25 identity blocks at partition offsets 0 and 32 (PE-transpose
    # requires the identity operand at the same base partition as the input)
    id32 = np.zeros((128, 128), np.float32)
    for b in range(B_PC):
        id32[b * 32:b * 32 + 25, 0:25] = np.eye(25)

    xpad16 = np.pad(x_feat, ((0, 0), (0, 0), (2, 2), (2, 2)),
                    mode="reflect").astype(np.float16)

    # conv1 im2col, zero pad 1: [54, H*W] per core
    rawpad = np.pad(raw_img, ((0, 0), (0, 0), (1, 1), (1, 1))).astype(np.float32)

    # wconv[b*27 + (c*9+i*3+j), b*64+o] = w1f[o, c, i, j]
    wconv = np.zeros((54, 128), np.float32)
    w_flat = w1f.transpose(1, 2, 3, 0).reshape(27, 64)  # (c*9+i*3+j, o)
    for b in range(B_PC):
        wconv[b * 27:(b + 1) * 27, b * 64:(b + 1) * 64] = w_flat
    wconv16 = wconv.astype(np.float16)

    in_maps = []
    for core in range(n_cores):
        bs = core * B_PC
        im2col = np.empty((54, H * W), np.float32)
        for b in range(B_PC):
            for c in range(3):
                for i in range(3):
                    for j in range(3):
                        p = b * 27 + c * 9 + i * 3 + j
                        im2col[p] = rawpad[bs + b, c, i:i + H, j:j + W].reshape(-1)
        in_maps.append({
            "x": xpad16[bs:bs + B_PC],
            "im2col": im2col.astype(np.float16),
            "wconv": wconv16,
            "b1r": b1r,
            "wft": wft16,
            "ident": ident,
            "id32": id32,
        })
    return in_maps


def run(inputs, trace=False, dve_splits=(7, 6), n_gp=0):
    x_feat = inputs["x_feat"]
    B, _, H, W = x_feat.shape
    nc = get_program(H, W, dve_splits, n_gp)
    in_maps = host_prep(**inputs)
    n_cores = len(in_maps)
    res = run_bass_kernel_spmd(nc, in_maps, list(range(n_cores)), trace=trace)
    out = np.concatenate([r["out"] for r in res.results], axis=0)
    return out, res


def kernel(**inputs) -> np.ndarray:
    out, _ = run(inputs, trace=False)
    return out
